# revision 1
# baseline (speedup 1.0000x reference)
"""Trainium2 Bass kernel for nn_ExpertFFN (top-1 MoE, B=4 S=2048 H=1024 E=8).

Strategy: shard tokens (batch*seq = 8192) across 8 NeuronCores, 1024 tokens
per core, with a load-balancing shard: each expert's tokens are dealt
round-robin across cores so every (core, expert) group is ~n_e/8 and the
static per-expert capacity can be small.  Router and expert weights are
replicated.  Per core:

  1. load x token-major, PE-transpose to feature-major (per token tile)
  2. fp32 router matmul + softmax (top-1 gate = 1/sum(exp(l - max)), onehot
     via is_equal against the row max)
  3. per-tile incremental slot assignment (PSUM cumsum matmuls + DVE prefix):
       slot(t) = cumsum_tile(t,e) - 1 + tile_base(tile,e) + CAP*e  @ e=argmax
  4. forward-scatter the gate-scaled x rows into a slot-ordered DRAM buffer
     (dispatch), scatter token ids into a DRAM inverse-permutation table
     (combine-time), sentinel 9999 in empty slots
  5. per expert e: contiguous staging loads, fp32 PE-transpose to
     feature-major, hi/lo bf16 split on DVE, grouped GEMM as 3-term bf16
     decomposition   x*w ~= x_hi*w_hi + x_lo*w_hi + x_hi*w_lo   with fp32
     PSUM accumulation and host-pre-split weights, fp32 PE-transpose back
     to token-major, indirect-scatter rows to y (bounds skips empty slots)

Expert weights stream on the sync DMA queue; index/staging traffic uses the
scalar/gpsimd queues so weight prefetch is never head-of-line blocked.
"""

import os
import sys

for _p in ("/opt/trn_rl_repo",):
    if _p not in sys.path:
        sys.path.insert(0, _p)

import numpy as np

P = 128
H = 1024
E = 8
TPC = 1024          # tokens per core
NCORES = 8
KC = H // P         # contraction chunks
MC = H // P         # output feature chunks
NTT = TPC // P      # token tiles per core
SENTINEL = 9999
PREC = os.environ.get("MOE_PREC", "hilo3")   # hilo3 | hilo4 | fp32
DEFAULT_CAP = 136   # balanced shard keeps every (core, expert) group <= this


def _build(router_bias: bool, expert_bias: bool, cap: int = DEFAULT_CAP,
           prec: str = PREC):
    import concourse.bass as bass
    import concourse.mybir as mybir
    import concourse.tile as tile
    from concourse import bacc
    from concourse.masks import make_identity, make_upper_triangular

    f32 = mybir.dt.float32
    bf16 = mybir.dt.bfloat16
    i32 = mybir.dt.int32
    AX = mybir.AxisListType
    OP = mybir.AluOpType
    ACT = mybir.ActivationFunctionType
    hilo = prec.startswith("hilo")
    four_term = prec == "hilo4"
    # fold the gate into x before dispatch when no expert bias; the
    # combine-time alternative (indirect gate gathers) measured slower
    prescale = not expert_bias
    CAP = cap
    CAPA, CAPB = P, CAP - P
    NSLOT = E * CAP

    nc = bacc.Bacc("TRN2", target_bir_lowering=False, debug=False,
                   num_devices=NCORES)

    x_d = nc.dram_tensor("x", [TPC, H], f32, kind="ExternalInput")
    rw_d = nc.dram_tensor("router_w", [H, E], f32, kind="ExternalInput")
    rb_d = nc.dram_tensor("router_b", [E], f32, kind="ExternalInput")
    if hilo:
        ewh_d = nc.dram_tensor("ew_hi", [E, H, H], bf16, kind="ExternalInput")
        ewl_d = nc.dram_tensor("ew_lo", [E, H, H], bf16, kind="ExternalInput")
    else:
        ew_d = nc.dram_tensor("expert_w", [E, H, H], f32,
                              kind="ExternalInput")
    eb_d = nc.dram_tensor("expert_b", [E, H], f32, kind="ExternalInput")
    y_d = nc.dram_tensor("y", [TPC, H], f32, kind="ExternalOutput")

    with tile.TileContext(nc) as tc:
        with (
            tc.tile_pool(name="consts", bufs=1) as cpool,
            tc.tile_pool(name="dram", bufs=1, space="DRAM") as dpool,
            tc.tile_pool(name="wload", bufs=2 * KC) as wpool,
            tc.tile_pool(name="slots", bufs=NTT) as slpool,
        ):
            # HAM warmup: sustained dummy matmul activity while the input
            # DMAs land, so the PE clock gate opens before the router runs
            warm = cpool.tile([P, P], bf16)
            nc.gpsimd.memset(warm[:], 0.0)
            with tc.tile_pool(name="warmps", bufs=1, space="PSUM") as wps:
                pw = wps.tile([P, P], f32, tag="pw", space="PSUM")
                for i in range(24):
                    nc.tensor.matmul(out=pw[:], lhsT=warm[:], rhs=warm[:],
                                     start=(i == 0), stop=(i == 23))

            id128 = cpool.tile([P, P], f32)
            make_identity(nc, id128[:])
            # 0/1 matrices and counts <= CAP are exact in bf16
            lt128 = cpool.tile([P, P], bf16)
            make_upper_triangular(nc, lt128[:], val=1.0, diag=True)
            ones_1x = cpool.tile([1, P], f32)
            nc.gpsimd.memset(ones_1x[:], 1.0)
            ones128 = cpool.tile([P, P], bf16)
            nc.gpsimd.memset(ones128[:], 1.0)
            ones_cap = cpool.tile([1, CAP], f32)
            nc.gpsimd.memset(ones_cap[:], 1.0)
            ecm1_i = cpool.tile([P, E], i32)
            nc.gpsimd.iota(ecm1_i[:], pattern=[[CAP, E]], base=-1,
                           channel_multiplier=0)
            tid_all = cpool.tile([P, NTT], i32)
            nc.gpsimd.iota(tid_all[:], pattern=[[P, NTT]], base=0,
                           channel_multiplier=1)
            sent = cpool.tile([1, NSLOT], i32)
            nc.gpsimd.memset(sent[:], SENTINEL)

            gidx_dram = dpool.tile([NSLOT, 1], i32)
            nc.gpsimd.dma_start(out=gidx_dram[:], in_=sent[:])
            xs_dram = dpool.tile([NSLOT, H], f32)
            if not prescale:
                gate_dram = dpool.tile([TPC, 1], f32)

            # expert weights on the sync queue: experts 0-1 up front, rest
            # two experts ahead inside the main loop
            def load_w(e, eng):
                if hilo:
                    whs, wls = [], []
                    for k in range(KC):
                        whk = wpool.tile([P, H], bf16, tag="wh")
                        eng.dma_start(
                            out=whk[:], in_=ewh_d[e, k * P:(k + 1) * P, :])
                        whs.append(whk)
                        wlk = wpool.tile([P, H], bf16, tag="wl")
                        eng.dma_start(
                            out=wlk[:], in_=ewl_d[e, k * P:(k + 1) * P, :])
                        wls.append(wlk)
                    return (whs, wls)
                ws = []
                for k in range(KC):
                    wk = wpool.tile([P, H], f32, tag="w")
                    eng.dma_start(
                        out=wk[:], in_=ew_d[e, k * P:(k + 1) * P, :])
                    ws.append(wk)
                return ws

            w_tiles = {e: load_w(e, nc.sync) for e in range(2)}

            # ---------------- phase 1: router + slot assignment ----------
            with (
                tc.tile_pool(name="rsb", bufs=NTT) as rpool,
                tc.tile_pool(name="rsmall", bufs=NTT) as spool,
                tc.tile_pool(name="rps", bufs=2, space="PSUM") as rpsum,
                tc.tile_pool(name="cps", bufs=2, space="PSUM") as cpsum,
                tc.tile_pool(name="cps1", bufs=2, space="PSUM") as cpsum1,
            ):
                xtm = []
                for t in range(NTT):
                    xt = rpool.tile([P, H], f32, tag="xtm")
                    eng = nc.scalar if t % 2 == 0 else nc.gpsimd
                    eng.dma_start(out=xt[:], in_=x_d[t * P:(t + 1) * P, :])
                    xtm.append(xt)
                rw_sb = []
                for k in range(KC):
                    rwk = spool.tile([P, E], f32, tag="rw")
                    nc.scalar.dma_start(out=rwk[:],
                                        in_=rw_d[k * P:(k + 1) * P, :])
                    rw_sb.append(rwk)
                if router_bias:
                    rb_sb = spool.tile([1, E], f32, tag="rb")
                    nc.scalar.dma_start(out=rb_sb[:], in_=rb_d[None, :])

                oh_all = rpool.tile([P, NTT * E], f32, tag="ohall")
                gate = []
                xsc = []
                tbacc = spool.tile([P, E], f32, tag="tbacc")
                nc.vector.tensor_copy(out=tbacc[:], in_=ecm1_i[:])
                slot_is = []
                for t in range(NTT):
                    pxt = rpsum.tile([P, H], f32, tag="pxt", space="PSUM")
                    for k in range(KC):
                        nc.tensor.transpose(
                            out=pxt[:, k * P:(k + 1) * P],
                            in_=xtm[t][:, k * P:(k + 1) * P],
                            identity=id128[:])
                    xTt = rpool.tile([P, H], f32, tag="xTt")
                    nc.vector.tensor_copy(out=xTt[:], in_=pxt[:])

                    plg = cpsum.tile([P, E], f32, tag="plg", space="PSUM")
                    for k in range(KC):
                        nc.tensor.matmul(
                            out=plg[:], lhsT=xTt[:, k * P:(k + 1) * P],
                            rhs=rw_sb[k][:], start=(k == 0),
                            stop=(k == KC - 1 and not router_bias))
                    if router_bias:
                        nc.tensor.matmul(out=plg[:], lhsT=ones_1x[:],
                                         rhs=rb_sb[:], start=False, stop=True)

                    negm = spool.tile([P, 1], f32, tag="negm")
                    nc.vector.tensor_reduce(out=negm[:], in_=plg[:], axis=AX.X,
                                            op=OP.max, negate=True)
                    m_t = spool.tile([P, 1], f32, tag="m")
                    nc.vector.tensor_scalar_mul(out=m_t[:], in0=negm[:],
                                                scalar1=-1.0)
                    esum = spool.tile([P, 1], f32, tag="esum")
                    etmp = spool.tile([P, E], f32, tag="etmp")
                    nc.scalar.activation(out=etmp[:], in_=plg[:], func=ACT.Exp,
                                         bias=negm[:], scale=1.0,
                                         accum_out=esum[:])
                    g_t = spool.tile([P, 1], f32, tag="gate")
                    nc.vector.reciprocal(out=g_t[:], in_=esum[:])
                    gate.append(g_t)
                    nc.vector.tensor_scalar(
                        out=oh_all[:, t * E:(t + 1) * E], in0=plg[:],
                        scalar1=m_t[:], scalar2=None, op0=OP.is_equal)
                    if prescale:
                        xs_t = rpool.tile([P, H], f32, tag="xsc")
                        nc.vector.tensor_scalar(out=xs_t[:], in0=xtm[t][:],
                                                scalar1=g_t[:], scalar2=None,
                                                op0=OP.mult)
                        xsc.append(xs_t)
                    else:
                        xsc.append(xtm[t])
                        nc.scalar.dma_start(
                            out=gate_dram[t * P:(t + 1) * P, :], in_=g_t[:])

                    # incremental slot computation; this tile's dispatch
                    # scatter fires as soon as softmax + running prefix land
                    blk = slice(t * E, (t + 1) * E)
                    oh_bf = spool.tile([P, E], bf16, tag="ohbf")
                    nc.vector.tensor_copy(out=oh_bf[:], in_=oh_all[:, blk])
                    pcc = cpsum1.tile([P, 2 * E], f32, tag="pcc",
                                      space="PSUM")
                    nc.tensor.matmul(out=pcc[:, 0:E], lhsT=lt128[:],
                                     rhs=oh_bf[:], start=True,
                                     stop=True)
                    nc.tensor.matmul(out=pcc[:, E:2 * E], lhsT=ones128[:],
                                     rhs=oh_bf[:], start=True,
                                     stop=True)
                    tmp = spool.tile([P, E], f32, tag="tmp")
                    nc.vector.tensor_tensor(out=tmp[:], in0=pcc[:, 0:E],
                                            in1=tbacc[:], op=OP.add)
                    junk = spool.tile([P, E], f32, tag="junk")
                    nc.vector.tensor_tensor(out=junk[:], in0=tmp[:],
                                            in1=oh_all[:, blk], op=OP.mult)
                    slot_f = spool.tile([P, 1], f32, tag="slotf")
                    nc.vector.tensor_reduce(out=slot_f[:], in_=junk[:],
                                            axis=AX.X, op=OP.add)
                    slot_i = slpool.tile([P, 1], i32, tag="sloti")
                    nc.vector.tensor_copy(out=slot_i[:], in_=slot_f[:])
                    slot_is.append(slot_i)
                    if t < NTT - 1:
                        nc.vector.tensor_tensor(out=tbacc[:], in0=tbacc[:],
                                                in1=pcc[:, E:2 * E],
                                                op=OP.add)
                    nc.gpsimd.indirect_dma_start(
                        out=xs_dram[:],
                        out_offset=bass.IndirectOffsetOnAxis(
                            ap=slot_i[:, :1], axis=0),
                        in_=xsc[t][:], in_offset=None)

            # ---------------- phase 2: per-expert grouped GEMM ------------
            with (
                tc.tile_pool(name="est", bufs=4) as stpool,
                tc.tile_pool(name="exs", bufs=2 * KC) as xspool,
                tc.tile_pool(name="eyt", bufs=2 * MC) as ytpool,
                tc.tile_pool(name="eysb", bufs=3) as ypool,
                tc.tile_pool(name="egi", bufs=E) as gipool,
                tc.tile_pool(name="exps", bufs=2, space="PSUM") as xpsum,
                tc.tile_pool(name="eyps", bufs=2, space="PSUM") as ypsum,
                tc.tile_pool(name="etps", bufs=2, space="PSUM") as tpsum,
            ):
                # staging-B for the first two experts ahead of the
                # tid-scatters so they aren't stuck behind them in the
                # gpsimd FIFO
                stB_pre = {}
                for e in range(2):
                    base = e * CAP
                    stB = stpool.tile([CAPB, H], f32, tag="stB")
                    nc.gpsimd.dma_start(
                        out=stB[:], in_=xs_dram[base + CAPA:base + CAP, :])
                    stB_pre[e] = stB
                # inverse permutation (combine-time): gidx[slot] = token
                for t in range(NTT):
                    nc.gpsimd.indirect_dma_start(
                        out=gidx_dram[:],
                        out_offset=bass.IndirectOffsetOnAxis(
                            ap=slot_is[t][:, :1], axis=0),
                        in_=tid_all[:, t:t + 1], in_offset=None)
                # index readbacks on the scalar queue (combine time)
                gAB = []
                for e in range(E):
                    base = e * CAP
                    gA = gipool.tile([CAPA, 1], i32, tag="gA")
                    nc.scalar.dma_start(out=gA[:],
                                        in_=gidx_dram[base:base + CAPA, :])
                    gB = gipool.tile([CAPB, 1], i32, tag="gB")
                    nc.scalar.dma_start(
                        out=gB[:], in_=gidx_dram[base + CAPA:base + CAP, :])
                    gAB.append((gA, gB))

                for e in range(E):
                    gA, gB = gAB[e]
                    base = e * CAP
                    # load in H-halves so the first transposes start earlier
                    stA = stpool.tile([CAPA, H], f32, tag="stA")
                    nc.sync.dma_start(
                        out=stA[:, 0:H // 2],
                        in_=xs_dram[base:base + CAPA, 0:H // 2])
                    nc.sync.dma_start(
                        out=stA[:, H // 2:H],
                        in_=xs_dram[base:base + CAPA, H // 2:H])
                    if e < 2:
                        stB = stB_pre[e]
                    else:
                        stB = stpool.tile([CAPB, H], f32, tag="stB")
                        nc.gpsimd.dma_start(
                            out=stB[:], in_=xs_dram[base + CAPA:base + CAP, :])
                    if e + 2 < E:
                        w_tiles[e + 2] = load_w(e + 2, nc.scalar)

                    # fp32 transpose to feature-major, then hi/lo split on
                    # the (idle) vector engine into one packed [hi|lo] tile
                    if hilo:
                        xsp = []
                        for k in range(KC):
                            ks = slice(k * P, (k + 1) * P)
                            pxs = xpsum.tile([P, CAP], f32, tag="pxs",
                                             space="PSUM")
                            nc.tensor.transpose(out=pxs[:, 0:CAPA],
                                                in_=stA[:, ks],
                                                identity=id128[:])
                            nc.tensor.transpose(out=pxs[:, CAPA:CAP],
                                                in_=stB[:, ks],
                                                identity=id128[:CAPB, :CAPB])
                            xspk = xspool.tile([P, 2 * CAP], bf16, tag="xsp")
                            nc.vector.tensor_copy(out=xspk[:, 0:CAP],
                                                  in_=pxs[:])
                            xshf = xspool.tile([P, CAP], f32, tag="xshf")
                            nc.vector.tensor_copy(out=xshf[:],
                                                  in_=xspk[:, 0:CAP])
                            nc.vector.tensor_tensor(out=xspk[:, CAP:2 * CAP],
                                                    in0=pxs[:], in1=xshf[:],
                                                    op=OP.subtract)
                            xsp.append(xspk)
                        wh_sb, wl_sb = w_tiles[e]
                    else:
                        xs = []
                        for k in range(KC):
                            ks = slice(k * P, (k + 1) * P)
                            pxs = xpsum.tile([P, CAP], f32, tag="pxs",
                                             space="PSUM")
                            nc.tensor.transpose(out=pxs[:, 0:CAPA],
                                                in_=stA[:, ks],
                                                identity=id128[:])
                            nc.tensor.transpose(out=pxs[:, CAPA:CAP],
                                                in_=stB[:, ks],
                                                identity=id128[:CAPB, :CAPB])
                            xsk = xspool.tile([P, CAP], f32, tag="xs")
                            nc.vector.tensor_copy(out=xsk[:], in_=pxs[:])
                            xs.append(xsk)
                        w_sb = w_tiles[e]

                    if expert_bias:
                        eb_sb = gipool.tile([1, H], f32, tag="eb")
                        nc.scalar.dma_start(out=eb_sb[:], in_=eb_d[e, None, :])

                    yt = []
                    for m in range(MC):
                        ms = slice(m * P, (m + 1) * P)
                        ytm = ytpool.tile([P, CAP], f32, tag="yt")
                        if hilo:
                            pyt = ypsum.tile([P, CAP], f32, tag="pyt",
                                             space="PSUM")
                            for k in range(KC):
                                last = (k == KC - 1 and not expert_bias)
                                nc.tensor.matmul(
                                    out=pyt[:], lhsT=wh_sb[k][:, ms],
                                    rhs=xsp[k][:, 0:CAP], start=(k == 0),
                                    stop=False)
                                nc.tensor.matmul(
                                    out=pyt[:], lhsT=wh_sb[k][:, ms],
                                    rhs=xsp[k][:, CAP:2 * CAP], start=False,
                                    stop=False)
                                nc.tensor.matmul(
                                    out=pyt[:], lhsT=wl_sb[k][:, ms],
                                    rhs=xsp[k][:, 0:CAP], start=False,
                                    stop=(last and not four_term))
                                if four_term:
                                    nc.tensor.matmul(
                                        out=pyt[:], lhsT=wl_sb[k][:, ms],
                                        rhs=xsp[k][:, CAP:2 * CAP],
                                        start=False, stop=last)
                            if expert_bias:
                                nc.tensor.matmul(
                                    out=pyt[:], lhsT=eb_sb[:, ms],
                                    rhs=ones_cap[:], start=False, stop=True)
                            nc.vector.tensor_copy(out=ytm[:], in_=pyt[:])
                        else:
                            pyt = ypsum.tile([P, CAP], f32, tag="pytf",
                                             space="PSUM")
                            for k in range(KC):
                                nc.tensor.matmul(
                                    out=pyt[:], lhsT=w_sb[k][:, ms],
                                    rhs=xs[k][:], start=(k == 0),
                                    stop=(k == KC - 1 and not expert_bias))
                            if expert_bias:
                                nc.tensor.matmul(
                                    out=pyt[:], lhsT=eb_sb[:, ms],
                                    rhs=ones_cap[:], start=False, stop=True)
                            nc.vector.tensor_copy(out=ytm[:], in_=pyt[:])
                        yt.append(ytm)

                    if not prescale:
                        gsA = gipool.tile([CAPA, 1], f32, tag="gsA")
                        nc.gpsimd.indirect_dma_start(
                            out=gsA[:], out_offset=None, in_=gate_dram[:],
                            in_offset=bass.IndirectOffsetOnAxis(ap=gA[:, :1],
                                                                axis=0),
                            bounds_check=TPC - 1, oob_is_err=False)
                        gsB = gipool.tile([CAPB, 1], f32, tag="gsB")
                        nc.gpsimd.indirect_dma_start(
                            out=gsB[:], out_offset=None, in_=gate_dram[:],
                            in_offset=bass.IndirectOffsetOnAxis(ap=gB[:, :1],
                                                                axis=0),
                            bounds_check=TPC - 1, oob_is_err=False)

                    ptokA = tpsum.tile([P, H], f32, tag="ptok", space="PSUM")
                    for m in range(MC):
                        ms = slice(m * P, (m + 1) * P)
                        nc.tensor.transpose(out=ptokA[:, ms],
                                            in_=yt[m][:, 0:CAPA],
                                            identity=id128[:])
                    yA = ypool.tile([CAPA, H], f32, tag="yA")
                    if prescale:
                        nc.vector.tensor_copy(out=yA[:], in_=ptokA[:])
                    else:
                        nc.vector.tensor_scalar(out=yA[:], in0=ptokA[:],
                                                scalar1=gsA[:], scalar2=None,
                                                op0=OP.mult)
                    nc.gpsimd.indirect_dma_start(
                        out=y_d[:],
                        out_offset=bass.IndirectOffsetOnAxis(ap=gA[:, :1],
                                                            axis=0),
                        in_=yA[:], in_offset=None,
                        bounds_check=TPC - 1, oob_is_err=False)

                    ptokB = tpsum.tile([P, H], f32, tag="ptok", space="PSUM")
                    for m in range(MC):
                        ms = slice(m * P, (m + 1) * P)
                        nc.tensor.transpose(out=ptokB[0:CAPB, ms],
                                            in_=yt[m][:, CAPA:CAP],
                                            identity=id128[:])
                    yB = ypool.tile([CAPB, H], f32, tag="yB")
                    if prescale:
                        nc.vector.tensor_copy(out=yB[:], in_=ptokB[0:CAPB, :])
                    else:
                        nc.vector.tensor_scalar(out=yB[:], in0=ptokB[0:CAPB, :],
                                                scalar1=gsB[:], scalar2=None,
                                                op0=OP.mult)
                    nc.gpsimd.indirect_dma_start(
                        out=y_d[:],
                        out_offset=bass.IndirectOffsetOnAxis(ap=gB[:, :1],
                                                            axis=0),
                        in_=yB[:], in_offset=None,
                        bounds_check=TPC - 1, oob_is_err=False)

    nc.compile()
    return nc


_NC_CACHE = {}


def _get_nc(router_bias: bool, expert_bias: bool, cap: int = DEFAULT_CAP,
            prec: str = PREC):
    key = (router_bias, expert_bias, cap, prec)
    if key not in _NC_CACHE:
        _NC_CACHE[key] = _build(*key)
    return _NC_CACHE[key]


def _split_hilo(w):
    import ml_dtypes
    hi = w.astype(ml_dtypes.bfloat16)
    lo = (w - hi.astype(np.float32)).astype(ml_dtypes.bfloat16)
    return np.ascontiguousarray(hi), np.ascontiguousarray(lo)


def balanced_perm(eidx):
    """Token permutation dealing each expert's tokens across cores so every
    (core, expert) group is ~n_e/NCORES and core totals are exactly TPC."""
    T = eidx.shape[0]
    groups = [np.where(eidx == e)[0] for e in range(E)]
    counts = np.zeros((NCORES, E), dtype=np.int64)
    for e in range(E):
        n = len(groups[e])
        base, rem = divmod(n, NCORES)
        counts[:, e] = base
        # give the remainder to the currently least-loaded cores
        order = np.argsort(counts.sum(1), kind="stable")
        counts[order[:rem], e] += 1
    # fix core totals to exactly TPC by moving single tokens
    totals = counts.sum(1)
    while True:
        hi_c = int(np.argmax(totals))
        lo_c = int(np.argmin(totals))
        if totals[hi_c] <= TPC and totals[lo_c] >= TPC:
            break
        moved = False
        for e in np.argsort(-counts[hi_c]):
            if counts[hi_c, e] > 0:
                counts[hi_c, e] -= 1
                counts[lo_c, e] += 1
                totals[hi_c] -= 1
                totals[lo_c] += 1
                moved = True
                break
        assert moved
    assert (counts.sum(1) == TPC).all()
    # build per-core token lists following the counts
    taken = [0] * E
    core_tokens = []
    for c in range(NCORES):
        toks = []
        for e in range(E):
            k = counts[c, e]
            toks.append(groups[e][taken[e]:taken[e] + k])
            taken[e] += k
        core_tokens.append(np.concatenate(toks))
    perm = np.concatenate(core_tokens)
    assert perm.shape == (T,) and len(np.unique(perm)) == T
    return perm, int(counts.max())


def plan(x, router_w, router_b):
    """Host-side shard plan: balanced permutation + capacity."""
    logits = x.reshape(-1, H) @ router_w + router_b
    eidx = logits.argmax(-1)
    perm, maxcell = balanced_perm(eidx)
    cap = max(DEFAULT_CAP, ((maxcell + 7) // 8) * 8)
    return perm, cap


def make_in_maps(x, router_w, router_b, expert_w, expert_b, perm,
                 prec=PREC):
    xt = x.reshape(-1, H)[perm].reshape(NCORES, TPC, H)
    base = {"router_w": router_w, "router_b": router_b, "expert_b": expert_b}
    if prec.startswith("hilo"):
        hi, lo = _split_hilo(expert_w)
        base["ew_hi"] = hi
        base["ew_lo"] = lo
    else:
        base["expert_w"] = expert_w
    return [dict(base, x=np.ascontiguousarray(xt[c])) for c in range(NCORES)]


def kernel(x, router_w, router_b, expert_w, expert_b):
    from concourse.bass_utils import run_bass_kernel_spmd

    x = np.ascontiguousarray(np.asarray(x, dtype=np.float32))
    router_w = np.ascontiguousarray(np.asarray(router_w, dtype=np.float32))
    router_b = np.ascontiguousarray(np.asarray(router_b, dtype=np.float32))
    expert_w = np.ascontiguousarray(np.asarray(expert_w, dtype=np.float32))
    expert_b = np.ascontiguousarray(np.asarray(expert_b, dtype=np.float32))

    B, S, Hx = x.shape
    assert (B * S, Hx) == (NCORES * TPC, H), (x.shape,)

    perm, cap = plan(x, router_w, router_b)
    router_bias = bool(np.any(router_b != 0))
    expert_bias = bool(np.any(expert_b != 0))
    nc = _get_nc(router_bias, expert_bias, cap)

    in_maps = make_in_maps(x, router_w, router_b, expert_w, expert_b, perm)
    res = run_bass_kernel_spmd(nc, in_maps, list(range(NCORES)))
    y_perm = np.concatenate([res.results[c]["y"] for c in range(NCORES)],
                            axis=0)
    y = np.empty_like(y_perm)
    y[perm] = y_perm
    return y.reshape(B, S, H)



# revision 4
# speedup vs baseline: 4.4154x; 4.4154x over previous
"""Trainium2 Bass kernel for nn_ExpertFFN (top-1 MoE, B=4 S=2048 H=1024 E=8).

Strategy: EXPERT parallelism.  Core c owns expert c's weights only (bf16,
2 MB instead of 32 MB replicated), and the host does all routing:

  host:   logits = x @ router_w + router_b (fp32, same as the reference),
          idx = argmax, gate = softmax max = 1/sum(exp(l - max)).
          Tokens for expert e are gathered, pre-scaled by gate (y =
          gate*(x@W+b) = (gate*x)@W + gate*b), transposed to feature-major
          and cast to bf16, zero-padded to a shared capacity.
  device: pure single-expert GEMM  y[cap, H] = xT.T @ W  in bf16 with fp32
          PSUM accumulation.  Tokens ride the stationary operand (lhsT =
          xT column tile), weight columns stream as rhs (2 x 512-col PSUM
          banks per token tile).  The (k-chunk)-major loop over groups of
          4 token tiles keeps ~1.7us of PE work per arriving 0.5 MB chunk
          pair so compute hides the 4.2 MB input stream.
  host:   scatter y rows back by token index, add gate*expert_b, unshard.

Pure-bf16 precision measured at rel err 2.3e-3 vs the fp32 reference
(tolerance 2e-2).
"""

import sys

for _p in ("/opt/trn_rl_repo",):
    if _p not in sys.path:
        sys.path.insert(0, _p)

import numpy as np

P = 128
H = 1024
E = 8
NCORES = 8
KC = H // P          # contraction chunks
NB = H // 512        # 512-col PSUM bank slices of the output features
GRP = 4              # token tiles per PSUM group (4 tiles x 2 banks = 8)


def _build(cap: int):
    import concourse.mybir as mybir
    import concourse.tile as tile
    from concourse import bacc

    f32 = mybir.dt.float32
    bf16 = mybir.dt.bfloat16

    ntt = (cap + P - 1) // P

    nc = bacc.Bacc("TRN2", target_bir_lowering=False, debug=False,
                   num_devices=NCORES)

    w_d = nc.dram_tensor("w", [H, H], bf16, kind="ExternalInput")
    xt_d = nc.dram_tensor("xt", [H, cap], bf16, kind="ExternalInput")
    y_d = nc.dram_tensor("y", [cap, H], f32, kind="ExternalOutput")

    with tile.TileContext(nc) as tc:
        with (
            tc.tile_pool(name="consts", bufs=1) as cpool,
            tc.tile_pool(name="wp", bufs=KC) as wpool,
            tc.tile_pool(name="xp", bufs=KC) as xpool,
            tc.tile_pool(name="yp", bufs=GRP + 2) as ypool,
            tc.tile_pool(name="ps", bufs=GRP, space="PSUM") as pspool,
        ):
            # HAM warmup: dummy matmul activity from t=0 so the PE clock
            # gate opens while the input DMAs land
            warm = cpool.tile([P, P], bf16)
            nc.gpsimd.memset(warm[:], 0.0)
            pw = pspool.tile([P, P], f32, tag="ps", space="PSUM")
            for i in range(16):
                nc.tensor.matmul(out=pw[:], lhsT=warm[:], rhs=warm[:],
                                 start=(i == 0), stop=(i == 15))

            # interleaved weight/activation chunk loads on the two HWDGE
            # rings so chunk k of both streams lands at ~the same time
            wt, xtt = [], []
            for k in range(KC):
                wk = wpool.tile([P, H], bf16, tag="w")
                nc.sync.dma_start(out=wk[:], in_=w_d[k * P:(k + 1) * P, :])
                wt.append(wk)
                xk = xpool.tile([P, cap], bf16, tag="x")
                nc.scalar.dma_start(out=xk[:], in_=xt_d[k * P:(k + 1) * P, :])
                xtt.append(xk)

            for g0 in range(0, ntt, GRP):
                tiles = range(g0, min(g0 + GRP, ntt))
                ps = {}
                for t in tiles:
                    ps[t] = pspool.tile([P, H], f32, tag="ps", space="PSUM",
                                        name=f"ps{t}")
                # k-major: one chunk pair feeds 2*len(tiles) matmuls
                for k in range(KC):
                    for t in tiles:
                        tw = min(P, cap - t * P)
                        for n in range(NB):
                            nc.tensor.matmul(
                                out=ps[t][0:tw, n * 512:(n + 1) * 512],
                                lhsT=xtt[k][:, t * P:t * P + tw],
                                rhs=wt[k][:, n * 512:(n + 1) * 512],
                                start=(k == 0), stop=(k == KC - 1))
                for t in tiles:
                    tw = min(P, cap - t * P)
                    yb = ypool.tile([P, H], f32, tag="y")
                    nc.vector.tensor_copy(out=yb[0:tw, :], in_=ps[t][0:tw, :])
                    nc.gpsimd.dma_start(out=y_d[t * P:t * P + tw, :],
                                        in_=yb[0:tw, :])

    nc.compile()
    return nc


_NC_CACHE = {}


def _get_nc(cap: int):
    if cap not in _NC_CACHE:
        _NC_CACHE[cap] = _build(cap)
    return _NC_CACHE[cap]


def plan(x, router_w, router_b):
    """Host-side routing: token lists per expert, gate values, capacity."""
    xt = x.reshape(-1, H)
    logits = xt @ router_w + router_b
    idx = logits.argmax(-1)
    mx = logits.max(-1)
    gate = 1.0 / np.exp(logits - mx[:, None]).sum(-1)
    toks = [np.where(idx == e)[0] for e in range(E)]
    cap = max(P, -(-max(len(t) for t in toks) // 64) * 64)
    return toks, gate.astype(np.float32), cap


def make_in_maps(x, expert_w, toks, gate, cap):
    import ml_dtypes
    bf = ml_dtypes.bfloat16
    xt = x.reshape(-1, H)
    maps = []
    for e in range(E):
        te = toks[e]
        xs = np.zeros((H, cap), dtype=bf)
        xs[:, :len(te)] = (xt[te] * gate[te, None]).T.astype(bf)
        maps.append({
            "w": np.ascontiguousarray(expert_w[e].astype(bf)),
            "xt": xs,
        })
    return maps


def assemble(results, toks, gate, expert_b, shape):
    T = shape[0] * shape[1]
    y = np.empty((T, H), dtype=np.float32)
    for e in range(E):
        te = toks[e]
        y[te] = results[e]["y"][:len(te)]
        if expert_b is not None:
            y[te] += gate[te, None] * expert_b[e][None, :]
    return y.reshape(shape)


def kernel(x, router_w, router_b, expert_w, expert_b):
    from concourse.bass_utils import run_bass_kernel_spmd

    x = np.ascontiguousarray(np.asarray(x, dtype=np.float32))
    router_w = np.ascontiguousarray(np.asarray(router_w, dtype=np.float32))
    router_b = np.ascontiguousarray(np.asarray(router_b, dtype=np.float32))
    expert_w = np.ascontiguousarray(np.asarray(expert_w, dtype=np.float32))
    expert_b = np.ascontiguousarray(np.asarray(expert_b, dtype=np.float32))

    B, S, Hx = x.shape
    assert Hx == H and B * S % NCORES == 0, (x.shape,)

    toks, gate, cap = plan(x, router_w, router_b)
    nc = _get_nc(cap)
    in_maps = make_in_maps(x, expert_w, toks, gate, cap)
    res = run_bass_kernel_spmd(nc, in_maps, list(range(NCORES)))
    eb = expert_b if np.any(expert_b != 0) else None
    return assemble(res.results, toks, gate, eb, (B, S, H))


# revision 5
# speedup vs baseline: 5.1114x; 1.1576x over previous
"""Trainium2 Bass kernel for nn_ExpertFFN (top-1 MoE, B=4 S=2048 H=1024 E=8).

Strategy: EXPERT parallelism.  Core c owns expert c's weights only (bf16,
2 MB instead of 32 MB replicated), and the host does all routing:

  host:   logits = x @ router_w + router_b (fp32, same as the reference),
          idx = argmax, gate = softmax max = 1/sum(exp(l - max)).
          Tokens for expert e are gathered, pre-scaled by gate (y =
          gate*(x@W+b) = (gate*x)@W + gate*b), transposed to feature-major,
          cast to bf16, zero-padded to a shared capacity, and laid out
          k-chunk-major ([128, KC*n]) so the device loads each stream with
          4 large contiguous DMAs (sizes 1/1/2/4 chunks: fine-grained at
          the front for early compute start, big at the back for
          bandwidth; 8 HWDGE DMAs in flight total avoids sem-lane reuse
          stalls).
  device: pure single-expert GEMM  y[cap, H] = xT.T @ W  in bf16 with fp32
          PSUM accumulation.  Tokens ride the stationary operand (lhsT =
          xT column tile), weight columns stream as rhs (2 x 512-col PSUM
          banks per token tile).  The first 4 token tiles run (k-chunk)-
          major so each arriving chunk feeds ~2us of PE work; remaining
          tiles run t-major so evacuation (vector/scalar halves, y halves
          DMA'd on the gpsimd + sync queues) hides behind the next tile's
          matmuls and the kernel tail is one half-tile evacuation.
  host:   scatter y rows back by token index, add gate*expert_b, unshard.

Pure-bf16 precision measured at rel err 2.3e-3 vs the fp32 reference
(tolerance 2e-2).
"""

import sys

for _p in ("/opt/trn_rl_repo",):
    if _p not in sys.path:
        sys.path.insert(0, _p)

import numpy as np

P = 128
H = 1024
E = 8
NCORES = 8
KC = H // P          # contraction chunks
GRP = 4              # token tiles in the k-major (DMA-overlap) group
SPLITS = (1, 1, 2, 4)  # chunks per input DMA, per stream


def _build(cap: int):
    import concourse.mybir as mybir
    import concourse.tile as tile
    from concourse import bacc

    f32 = mybir.dt.float32
    bf16 = mybir.dt.bfloat16

    ntt = (cap + P - 1) // P

    nc = bacc.Bacc("TRN2", target_bir_lowering=False, debug=False,
                   num_devices=NCORES)

    # host pre-arranged chunk-major layouts: [p, k*N + j] = src[k*128+p, j]
    w_d = nc.dram_tensor("w", [P, KC * H], bf16, kind="ExternalInput")
    xt_d = nc.dram_tensor("xt", [P, KC * cap], bf16, kind="ExternalInput")
    y_d = nc.dram_tensor("y", [cap, H], f32, kind="ExternalOutput")

    with tile.TileContext(nc) as tc:
        with (
            tc.tile_pool(name="consts", bufs=1) as cpool,
            tc.tile_pool(name="wp", bufs=len(SPLITS)) as wpool,
            tc.tile_pool(name="xp", bufs=len(SPLITS)) as xpool,
            tc.tile_pool(name="yp", bufs=GRP + 2) as ypool,
            tc.tile_pool(name="ps", bufs=GRP, space="PSUM") as pspool,
        ):
            # HAM warmup: dummy matmul activity from t=0 so the PE clock
            # gate opens while the input DMAs land
            warm = cpool.tile([P, 512], bf16)
            nc.gpsimd.memset(warm[:], 0.0)
            pw = pspool.tile([P, 512], f32, tag="ps", space="PSUM")
            NWARM = 20
            for i in range(NWARM):
                ncol = 128 if i < NWARM - 4 else 512
                nc.tensor.matmul(out=pw[:, 0:ncol], lhsT=warm[:, 0:128],
                                 rhs=warm[:, 0:ncol],
                                 start=(i == 0), stop=(i == NWARM - 1))

            # graduated input DMAs on the two HWDGE rings
            wq, xq, qof = [], [], []
            off = 0
            for s in SPLITS:
                wt = wpool.tile([P, s * H], bf16, tag=f"w{off}",
                                name=f"w{off}")
                nc.sync.dma_start(out=wt[:],
                                  in_=w_d[:, off * H:(off + s) * H])
                xt = xpool.tile([P, s * cap], bf16, tag=f"x{off}",
                                name=f"x{off}")
                nc.scalar.dma_start(out=xt[:],
                                    in_=xt_d[:, off * cap:(off + s) * cap])
                for _ in range(s):
                    wq.append(wt)
                    xq.append(xt)
                    qof.append(off)
                off += s

            def wk(k):
                return wq[k][:, (k - qof[k]) * H:(k - qof[k] + 1) * H]

            def xk(k):
                return xq[k][:, (k - qof[k]) * cap:(k - qof[k] + 1) * cap]

            ps = {}

            def evac(t):
                tw = min(P, cap - t * P)
                yb = ypool.tile([P, H], f32, tag="y", name=f"y{t}")
                nc.vector.tensor_copy(out=yb[0:tw, 0:512],
                                      in_=ps[t][0:tw, 0:512])
                nc.scalar.copy(out=yb[0:tw, 512:H], in_=ps[t][0:tw, 512:H])
                nc.gpsimd.dma_start(out=y_d[t * P:t * P + tw, 0:512],
                                    in_=yb[0:tw, 0:512])
                nc.sync.dma_start(out=y_d[t * P:t * P + tw, 512:H],
                                  in_=yb[0:tw, 512:H])

            # group A: k-major over the first GRP token tiles so each
            # arriving chunk feeds 8 matmuls
            ga = range(0, min(GRP, ntt))
            for t in ga:
                ps[t] = pspool.tile([P, H], f32, tag="ps", space="PSUM",
                                    name=f"ps{t}")
            for k in range(KC):
                for t in ga:
                    tw = min(P, cap - t * P)
                    for n in range(2):
                        nc.tensor.matmul(
                            out=ps[t][0:tw, n * 512:(n + 1) * 512],
                            lhsT=xk(k)[:, t * P:t * P + tw],
                            rhs=wk(k)[:, n * 512:(n + 1) * 512],
                            start=(k == 0), stop=(k == KC - 1))
            for t in ga:
                evac(t)

            # remaining tiles: t-major; evacuation hides behind the next
            # tile's matmuls
            for t in range(GRP, ntt):
                tw = min(P, cap - t * P)
                ps[t] = pspool.tile([P, H], f32, tag="ps", space="PSUM",
                                    name=f"ps{t}")
                for k in range(KC):
                    for n in range(2):
                        nc.tensor.matmul(
                            out=ps[t][0:tw, n * 512:(n + 1) * 512],
                            lhsT=xk(k)[:, t * P:t * P + tw],
                            rhs=wk(k)[:, n * 512:(n + 1) * 512],
                            start=(k == 0), stop=(k == KC - 1))
                evac(t)

    nc.compile()
    return nc


_NC_CACHE = {}


def _get_nc(cap: int):
    if cap not in _NC_CACHE:
        _NC_CACHE[cap] = _build(cap)
    return _NC_CACHE[cap]


def plan(x, router_w, router_b):
    """Host-side routing: token lists per expert, gate values, capacity."""
    xt = x.reshape(-1, H)
    logits = xt @ router_w + router_b
    idx = logits.argmax(-1)
    mx = logits.max(-1)
    gate = 1.0 / np.exp(logits - mx[:, None]).sum(-1)
    toks = [np.where(idx == e)[0] for e in range(E)]
    cap = max(P, -(-max(len(t) for t in toks) // 64) * 64)
    return toks, gate.astype(np.float32), cap


def make_in_maps(x, expert_w, toks, gate, cap):
    import ml_dtypes
    bf = ml_dtypes.bfloat16
    xt = x.reshape(-1, H)
    maps = []
    for e in range(E):
        te = toks[e]
        xs = np.zeros((KC, P, cap), dtype=bf)
        xs.reshape(H, cap)[:, :len(te)] = (xt[te] * gate[te, None]).T.astype(bf)
        w = expert_w[e].astype(bf).reshape(KC, P, H)
        maps.append({
            "w": np.ascontiguousarray(w.transpose(1, 0, 2).reshape(P, KC * H)),
            "xt": np.ascontiguousarray(
                xs.transpose(1, 0, 2).reshape(P, KC * cap)),
        })
    return maps


def assemble(results, toks, gate, expert_b, shape):
    T = shape[0] * shape[1]
    y = np.empty((T, H), dtype=np.float32)
    for e in range(E):
        te = toks[e]
        y[te] = results[e]["y"][:len(te)]
        if expert_b is not None:
            y[te] += gate[te, None] * expert_b[e][None, :]
    return y.reshape(shape)


def kernel(x, router_w, router_b, expert_w, expert_b):
    from concourse.bass_utils import run_bass_kernel_spmd

    x = np.ascontiguousarray(np.asarray(x, dtype=np.float32))
    router_w = np.ascontiguousarray(np.asarray(router_w, dtype=np.float32))
    router_b = np.ascontiguousarray(np.asarray(router_b, dtype=np.float32))
    expert_w = np.ascontiguousarray(np.asarray(expert_w, dtype=np.float32))
    expert_b = np.ascontiguousarray(np.asarray(expert_b, dtype=np.float32))

    B, S, Hx = x.shape
    assert Hx == H and B * S % NCORES == 0, (x.shape,)

    toks, gate, cap = plan(x, router_w, router_b)
    nc = _get_nc(cap)
    in_maps = make_in_maps(x, expert_w, toks, gate, cap)
    res = run_bass_kernel_spmd(nc, in_maps, list(range(NCORES)))
    eb = expert_b if np.any(expert_b != 0) else None
    return assemble(res.results, toks, gate, eb, (B, S, H))
